# revision 1
# baseline (speedup 1.0000x reference)
"""Trainium2 Bass kernel for nn_CustomLoss_47931835023913.

Computes: loss = mean_i( logsumexp(output[i,:]) - output[i, target[i]] )
                 + (epoch**-0.65)*64 + 0.01   if any(target==2 & argmax==3)

Strategy (data-parallel over 8 NeuronCores, batch-sharded):
  * Host-side layout prep only: each row of `output` is rotated so that
    column 0 holds output[i, target[i]] (the CE gather becomes a strided
    column sum) and, for rows with target==2, column 1 holds output[i, 3]
    (the argmax flag test becomes a compare against the row max, which is
    rotation-invariant). A uint8 mask of target==2 rides along. Inputs ship
    as bf16 (CE mean over 4.2M rows is insensitive to unbiased rounding).
    All O(B) loss arithmetic runs on the NeuronCores.
  * Device per 128x512x10 tile:
      - ScalarE: exp() into two bf16 half-tiles (classes 0-4 / 5-9).
      - TensorE: 10 accumulating identity matmuls sum the halves' columns
        into PSUM -> per-row sum(exp) in fp32 (partition-passthrough adds).
      - ScalarE: ln() of the PSUM row sums with accum_out -> per-tile
        partial sum of the logsumexp term; a strided Identity accum over
        rotated column 0 -> partial sum of gathered logits.
      - VectorE: pairwise-tree max (bf16 tensor_tensor, 2x mode) for the
        row max; flag partial = sum(mask * (e[:,1] >= rowmax)).
  * Host combines the 8 cores' [128, 3*T] accumulators in float64 and adds
    the epoch correction.

bf16 exp values only feed (a) ln(sum(exp)) -- unbiased rounding noise that
averages out over 4.2M rows -- and (b) the argmax compare, where round-to-
nearest monotonicity guarantees no false negatives on the any() flag.
"""

import numpy as np

B = 4194304          # batch rows
C = 10               # classes
NCORES = 8
P = 128              # SBUF partitions
R = B // NCORES      # rows per core            = 524288
RP = R // P          # rows per partition       = 4096
TN = 512             # tile rows per partition
T = RP // TN         # tiles per core           = 8
MMN = 512            # matmul free-dim slice (one PSUM bank)
# row chunks per partition: half-size first/last chunks shorten pipeline
# fill (first exp starts sooner) and drain (shorter post-exp tail)
CHUNKS = [256, 256] + [512] * (T - 2) + [256, 256]
NCH = len(CHUNKS)
NP = NCH // 2     # chunk pairs (one ln per pair)

_CACHE = {}

# all activation funcs this kernel uses live in this one table set, so pin
# every InstActivation to it -> exactly one LoadActFuncSet in the program
_ACT_SET = "natural_log_exp_and_others"


def _pin_act_tables():
    import concourse.bacc as bacc_mod

    if getattr(bacc_mod.get_activation_tables, "_pinned", False):
        return
    orig = bacc_mod.get_activation_tables

    def pinned(module_arch):
        tables = orig(module_arch)
        return {
            name: (funcs if name == _ACT_SET else set())
            for name, funcs in tables.items()
        }

    pinned._pinned = True
    bacc_mod.get_activation_tables = pinned


def _build_nc(repeat=1):
    import concourse.mybir as mybir
    from concourse.bacc import Bacc
    from concourse.tile import TileContext

    _pin_act_tables()

    A = mybir.AluOpType
    F = mybir.ActivationFunctionType
    f32 = mybir.dt.float32
    bf16 = mybir.dt.bfloat16

    nc = Bacc("TRN2")
    x_d = nc.dram_tensor("x", [P, RP * C], bf16, kind="ExternalInput")
    m2_d = nc.dram_tensor("m2", [P, RP], mybir.dt.uint8, kind="ExternalInput")
    out_d = nc.dram_tensor("out", [P, NP + 2 * NCH], f32, kind="ExternalOutput")
    import ml_dtypes

    ident_d = nc.inline_tensor(
        np.eye(P, dtype=ml_dtypes.bfloat16), name="ident"
    )

    with TileContext(nc) as tc:
        with (
            tc.tile_pool(name="persist", bufs=1) as pp,
            tc.tile_pool(name="io", bufs=5) as iop,
            tc.tile_pool(name="work", bufs=3) as wp,
            tc.tile_pool(name="ps", bufs=4, space="PSUM") as psp,
        ):
            # ident is tiny and needed early by PE: HWDGE, queued first.
            # m2 chunks ride the HWDGE queue between x tiles so the flag
            # ops get them just in time without delaying the first x tile.
            ident = pp.tile([P, P], bf16)
            m2_all = pp.tile([P, RP], mybir.dt.uint8)
            # lse accumulates per chunk-PAIR (one ln per pair), g/flag per chunk
            acc = pp.tile([P, NP + 2 * NCH], f32)

            starts = np.cumsum([0] + CHUNKS[:-1]).tolist()
            s_pair = None
            for k in range(NCH * repeat):
                k = k % NCH
                r0, nr = starts[k], CHUNKS[k]
                pair, second = divmod(k, 2)
                pair_rows = CHUNKS[2 * pair] + CHUNKS[2 * pair + 1]
                off = CHUNKS[2 * pair] if second else 0
                x_t = iop.tile([P, TN * C], bf16, tag="x", name="x_t")[:, : nr * C]
                nc.sync.dma_start(x_t[:], x_d[:, r0 * C : (r0 + nr) * C])
                nc.sync.dma_start(
                    m2_all[:, r0 : r0 + nr], m2_d[:, r0 : r0 + nr]
                )
                if k == 0:
                    nc.sync.dma_start(ident[:], ident_d[:])
                xv = x_t.rearrange("p (n c) -> p n c", c=C)

                e_all = wp.tile([P, TN * C], bf16, tag="e", name="e_all", bufs=5)[:, : nr * C]
                ev = e_all.rearrange("p (n c) -> p n c", c=C)
                nc.scalar.activation(ev, xv, F.Exp)

                # row sum of the 10 exp columns: accumulating identity
                # matmuls (partition passthrough), one PSUM bank per
                # 512-column group; a PSUM tile spans a pair of chunks
                if not second:
                    s_pair = psp.tile([P, 2 * TN], f32, tag="s", name="s_pair")
                s_ps = s_pair[:, off : off + nr]
                for g in range(-(-nr // MMN)):
                    rows = slice(g * MMN, min((g + 1) * MMN, nr))
                    for c in range(C):
                        nc.tensor.matmul(
                            s_ps[:, rows], ident[:], ev[:, rows, c],
                            start=(c == 0), stop=(c == C - 1),
                        )

                # row max: pairwise tree on raw bf16 x (2x mode on level 1);
                # exp is monotone so comparing x is the same as comparing e,
                # and this decouples the DVE chain from ScalarE's exp
                mx1 = wp.tile([P, TN * 5], bf16, tag="mx1", name="mx1")[:, : nr * 5]
                mx1v = mx1.rearrange("p (n c) -> p n c", c=5)
                nc.vector.tensor_tensor(mx1v, xv[:, :, 0:5], xv[:, :, 5:10], A.max)
                mx2 = wp.tile([P, TN * 2], bf16, tag="mx2", name="mx2")[:, : nr * 2]
                mx2v = mx2.rearrange("p (n c) -> p n c", c=2)
                nc.vector.tensor_tensor(mx2v, mx1v[:, :, 0:2], mx1v[:, :, 2:4], A.max)
                v = wp.tile([P, TN], f32, tag="v", name="v")[:, :nr]
                nc.vector.tensor_tensor(v[:], mx2v[:, :, 0], mx2v[:, :, 1], A.max)
                rmax = wp.tile([P, TN], f32, tag="rmax", name="rmax")[:, :nr]
                nc.vector.tensor_tensor(rmax[:], v[:], mx1v[:, :, 4], A.max)

                # partial sums: lse per pair (ACT), gathered logit (DVE),
                # flag (DVE)
                if second:
                    lse_scr = wp.tile(
                        [P, 2 * TN], f32, tag="lse_scr", name="lse_scr"
                    )[:, :pair_rows]
                    nc.scalar.activation(
                        lse_scr[:], s_pair[:, :pair_rows], F.Ln,
                        accum_out=acc[:, pair : pair + 1],
                    )
                g_scr = wp.tile([P, TN], f32, tag="g_scr", name="g_scr")[:, :nr]
                nc.vector.tensor_scalar(
                    g_scr[:], xv[:, :, 0], 1.0, 0.0, A.mult, A.add,
                    accum_out=acc[:, NP + k : NP + k + 1],
                )
                eq = wp.tile([P, TN], f32, tag="eq", name="eq")[:, :nr]
                nc.vector.tensor_tensor(eq[:], xv[:, :, 1], rmax[:], A.is_ge)
                f_scr = wp.tile([P, TN], f32, tag="f_scr", name="f_scr")[:, :nr]
                nc.vector.scalar_tensor_tensor(
                    f_scr[:], m2_all[:, r0 : r0 + nr], 1.0, eq[:],
                    A.mult, A.mult,
                    accum_out=acc[:, NP + NCH + k : NP + NCH + k + 1],
                )

            nc.sync.dma_start(out_d[:], acc[:])
    nc.finalize()
    return nc


def _get_nc():
    if "nc" not in _CACHE:
        _CACHE["nc"] = _build_nc()
    return _CACHE["nc"]


def _prep_inputs(x, t32):
    """Rotate each row so column 0 is the target logit; build target==2 mask."""
    import ml_dtypes

    idx = (t32[:, None] + np.arange(C, dtype=np.int32)[None, :]) % C
    xr = np.take_along_axis(x, idx, axis=1).astype(ml_dtypes.bfloat16)
    m2 = (t32 == 2).astype(np.uint8)
    xs = xr.reshape(NCORES, P, RP * C)
    ms = m2.reshape(NCORES, P, RP)
    return xs, ms


def kernel(output=None, target=None, epoch=None):
    from concourse import bass_utils

    x = np.asarray(output)
    if x.dtype != np.float32:
        x = x.astype(np.float32)
    t32 = np.asarray(target).astype(np.int32)
    ep = int(np.asarray(epoch))
    assert x.shape == (B, C) and t32.shape == (B,)

    xs, ms = _prep_inputs(x, t32)
    in_maps = [
        {"x": np.ascontiguousarray(xs[i]), "m2": np.ascontiguousarray(ms[i])}
        for i in range(NCORES)
    ]
    nc = _get_nc()
    res = bass_utils.run_bass_kernel_spmd(nc, in_maps, core_ids=list(range(NCORES)))

    lse_sum = 0.0
    g_sum = 0.0
    flg = 0.0
    for rmap in res.results:
        o = rmap["out"].astype(np.float64)
        lse_sum += o[:, 0:NP].sum()
        g_sum += o[:, NP : NP + NCH].sum()
        flg += o[:, NP + NCH : NP + 2 * NCH].sum()

    init_loss = (lse_sum - g_sum) / B
    corr = (float(ep) ** -0.65) / (4.0 ** -3) + 0.01
    loss = init_loss + (corr if flg > 0 else 0.0)
    return np.array(loss, dtype=np.float32)



# revision 2
# speedup vs baseline: 1.9363x; 1.9363x over previous
"""Trainium2 Bass kernel for nn_CustomLoss_47931835023913.

Computes: loss = mean_i( ln(sum_j exp(x_ij)) - x[i, target_i] )
                 + ((epoch**-0.65)*64 + 0.01) if any(target==2 & argmax==3)

v2 strategy (fp8 shipping, three-engine exp, DoubleRow row-sums):
  * Host-side LAYOUT prep only (rotation + dtype cast + compaction); all
    O(B) arithmetic runs on the NeuronCores.
      - rows rotated so column 0 holds x[i, target_i] (CE gather becomes
        a column sum); cast to fp8 e4m3 (CE mean over 4.2M rows gives a
        ~0.5 abs tolerance at rel 2e-2; e4m3 noise is zero-mean).
      - layout is tile-contiguous class-major [P, T, C, TN] so every
        engine reads/writes contiguous runs (the v1 kernel's strided
        matmul moving operands were the top bottleneck).
      - rows with target==2 are compacted into a separate small array
        (col 0 = x[i,3]) so the argmax flag only processes ~10% of rows
        and no mask ships.
  * Device per 512-row chunk:
      - exp into a shared e5m2 tile, split by contiguous class ranges
        across THREE engines: ScalarE native Exp, and DVE + GPSIMD via
        the Schraudolph bit trick (u8 = rint(x*4*log2e + 59.8) bitcast
        to e5m2 == 2^(x*log2e) with mean rel err +0.09%).
      - TensorE: 5 fp8 DoubleRow matmuls (stacked-identity weights) sum
        class pairs into one PSUM bank -> per-row sum(exp) in fp32.
      - TensorE: 1 plain fp8 matmul accumulates raw column 0 across all
        chunks into a persistent PSUM bank (the CE gather term).
      - ScalarE: Ln over a 2-chunk PSUM pair with accum_out.
  * Flag: DVE pairwise max tree over the compacted rows' cols 1..9,
    is_ge against col 0, accumulated count.
  * Host combines the 8 cores' [128, 6] accumulators in float64.
"""

import numpy as np

B = 4194304          # batch rows
C = 10               # classes
NCORES = 8
P = 128              # SBUF partitions
R = B // NCORES      # rows per core            = 524288
RP = R // P          # rows per partition       = 4096
TN = 512             # tile rows per partition
T = RP // TN         # chunks per core          = 8
NPAIR = T // 2

# exp class-range split points (bytes within a 5120-elem chunk):
# ACT [0, SPLIT_A), DVE [SPLIT_A, SPLIT_D), GPSIMD [SPLIT_D, 5120)
SPLIT_A = 1536       # 30% on ScalarE (it also runs the Lns)
SPLIT_D = 3072       # 30% on DVE (it also runs the flag tree)
                     # 40%... tuned below; GPSIMD gets the tail

FN = 416             # flag rows per partition; 8*128*416 = 425984 total
                     # capacity vs E[count]=419430, sd~614 (+10.7 sd)

SCH_A = float(4.0 * np.log2(np.e))
SCH_B = 59.8         # 60 (e5m2 bias*4) - 0.2 interp-bias centering

_CACHE = {}

_ACT_SET = "natural_log_exp_and_others"


def _pin_act_tables():
    import concourse.bacc as bacc_mod

    if getattr(bacc_mod.get_activation_tables, "_pinned", False):
        return
    orig = bacc_mod.get_activation_tables

    def pinned(module_arch):
        tables = orig(module_arch)
        return {
            name: (funcs if name == _ACT_SET else set())
            for name, funcs in tables.items()
        }

    pinned._pinned = True
    bacc_mod.get_activation_tables = pinned


def _build_nc():
    import ml_dtypes
    import concourse.mybir as mybir
    from concourse.bacc import Bacc
    from concourse.tile import TileContext

    _pin_act_tables()

    A = mybir.AluOpType
    F = mybir.ActivationFunctionType
    f32 = mybir.dt.float32
    e4 = mybir.dt.float8e4
    e5 = mybir.dt.float8e5
    u8 = mybir.dt.uint8
    CH = C * TN                       # elems per chunk per partition

    nc = Bacc("TRN2")
    x_d = nc.dram_tensor("x", [P, T * CH], e4, kind="ExternalInput")
    xf_d = nc.dram_tensor("xf", [P, C * FN], e4, kind="ExternalInput")
    out_d = nc.dram_tensor("out", [P, 6], f32, kind="ExternalOutput")

    # stacked identity for DoubleRow ([P, 2, P] as flat [P, 2P]) in e5m2,
    # plus a plain e4m3 identity for the raw-x gather matmul
    ident2_d = nc.inline_tensor(
        np.broadcast_to(
            np.eye(P, dtype=ml_dtypes.float8_e5m2)[:, None, :], (P, 2, P)
        ).reshape(P, 2 * P).copy(),
        name="ident2",
    )
    ident4_d = nc.inline_tensor(
        np.eye(P, dtype=ml_dtypes.float8_e4m3fn), name="ident4"
    )

    with TileContext(nc) as tc:
        with (
            tc.tile_pool(name="persist", bufs=1) as pp,
            tc.tile_pool(name="io", bufs=4) as iop,
            tc.tile_pool(name="work", bufs=4) as wp,
            tc.tile_pool(name="lnp", bufs=2) as lnp,
            tc.tile_pool(name="ps", bufs=3, space="PSUM") as psp,
            tc.tile_pool(name="psg", bufs=1, space="PSUM") as psgp,
        ):
            idt2 = pp.tile([P, 2 * P], e5)
            idt4 = pp.tile([P, P], e4)
            xf = pp.tile([P, C * FN], e4)
            acc = pp.tile([P, 6], f32)
            nc.sync.dma_start(idt2[:], ident2_d[:])
            nc.sync.dma_start(idt4[:], ident4_d[:])
            nc.sync.dma_start(xf[:], xf_d[:])
            idt2v = idt2.rearrange("p (a b) -> p a b", a=2)

            psg = psgp.tile([P, TN], f32, tag="g", name="psg")

            s_pair = None
            for t in range(T):
                x_t = iop.tile([P, CH], e4, tag="x", name="x_t")
                nc.sync.dma_start(x_t[:], x_d[:, t * CH : (t + 1) * CH])

                e_t = wp.tile([P, CH], u8, tag="e", name="e_t")
                e5v = e_t.bitcast(e5)
                # three-engine exp, contiguous splits
                nc.scalar.activation(
                    e5v[:, 0:SPLIT_A], x_t[:, 0:SPLIT_A], F.Exp
                )
                nc.vector.tensor_scalar(
                    e_t[:, SPLIT_A:SPLIT_D], x_t[:, SPLIT_A:SPLIT_D],
                    SCH_A, SCH_B, A.mult, A.add,
                )
                nc.gpsimd.tensor_scalar(
                    e_t[:, SPLIT_D:CH], x_t[:, SPLIT_D:CH],
                    SCH_A, SCH_B, A.mult, A.add,
                )

                # row sums: 5 DoubleRow matmuls accumulate class pairs
                pair, odd = divmod(t, 2)
                if not odd:
                    s_pair = psp.tile([P, 2 * TN], f32, tag="s", name="s_pair")
                s_ps = s_pair[:, odd * TN : (odd + 1) * TN]
                ev = e5v.rearrange("p (c n) -> p c n", c=C)
                for cc in range(C // 2):
                    nc.tensor.matmul(
                        s_ps, idt2v, ev[:, 2 * cc : 2 * cc + 2, :],
                        start=(cc == 0), stop=(cc == C // 2 - 1),
                        perf_mode=mybir.MatmulPerfMode.DoubleRow,
                        skip_group_check=True,
                    )

                # gather: accumulate raw column 0 across chunks (plain fp8)
                nc.tensor.matmul(
                    psg[:], idt4[:], x_t[:, 0:TN],
                    start=(t == 0), stop=(t == T - 1),
                    skip_group_check=True,
                )

                if odd:
                    lnscr = lnp.tile([P, 2 * TN], f32, tag="ln", name="lnscr")
                    nc.scalar.activation(
                        lnscr[:], s_pair[:], F.Ln,
                        accum_out=acc[:, pair : pair + 1],
                    )

            # flag: max tree over cols 1..9 of compacted rows, vs col 0
            xfv = xf.rearrange("p (c n) -> p c n", c=C)
            m1 = wp.tile([P, 4 * FN], e4, tag="m1", name="m1", bufs=1)
            m1v = m1.rearrange("p (c n) -> p c n", c=4)
            nc.vector.tensor_tensor(m1v, xfv[:, 1:5, :], xfv[:, 5:9, :], A.max)
            m2 = wp.tile([P, 2 * FN], e4, tag="m2", name="m2", bufs=1)
            m2v = m2.rearrange("p (c n) -> p c n", c=2)
            nc.vector.tensor_tensor(m2v, m1v[:, 0:2, :], m1v[:, 2:4, :], A.max)
            m3 = wp.tile([P, FN], e4, tag="m3", name="m3", bufs=1)
            nc.vector.tensor_tensor(m3[:], m2v[:, 0, :], m2v[:, 1, :], A.max)
            m4 = wp.tile([P, FN], e4, tag="m4", name="m4", bufs=1)
            nc.vector.tensor_tensor(m4[:], m3[:], xfv[:, 9, :], A.max)
            ge = wp.tile([P, FN], f32, tag="ge", name="ge", bufs=1)
            nc.vector.scalar_tensor_tensor(
                ge[:], xfv[:, 0, :], 1.0, m4[:], A.mult, A.is_ge,
                accum_out=acc[:, 4:5],
            )

            # gather total
            gscr = wp.tile([P, TN], f32, tag="gs", name="gscr", bufs=1)
            nc.vector.tensor_scalar(
                gscr[:], psg[:], 1.0, 0.0, A.mult, A.add,
                accum_out=acc[:, 5:6],
            )

            nc.sync.dma_start(out_d[:], acc[:])
    nc.finalize()
    return nc


def _get_nc():
    if "nc" not in _CACHE:
        _CACHE["nc"] = _build_nc()
    return _CACHE["nc"]


def _prep_inputs(x, t32):
    """Rotate rows by target, cast fp8, tile-contiguous class-major layout;
    compact target==2 rows (col 0 = x[:,3]) for the flag path."""
    import ml_dtypes

    idx = (t32[:, None] + np.arange(C, dtype=np.int32)[None, :]) % C
    xr = np.take_along_axis(x, idx, axis=1).astype(ml_dtypes.float8_e4m3fn)
    # [B, C] -> [cores, P, T, TN, C] -> [cores, P, T, C, TN]
    xs = np.ascontiguousarray(
        xr.reshape(NCORES, P, T, TN, C).transpose(0, 1, 2, 4, 3)
    ).reshape(NCORES, P, T * C * TN)

    fidx = np.flatnonzero(t32 == 2)
    nf_cap = NCORES * P * FN
    host_flag = False
    if len(fidx) > nf_cap:
        # overflow beyond device capacity: fold the excess on host
        # (never triggers for randn inputs; correctness backstop)
        extra = fidx[nf_cap:]
        host_flag = bool(
            np.any(np.argmax(x[extra], axis=1) == 3)
        )
        fidx = fidx[:nf_cap]
    xf_rows = x[fidx][:, [3, 4, 5, 6, 7, 8, 9, 0, 1, 2]].astype(
        ml_dtypes.float8_e4m3fn
    )
    pad = np.zeros((nf_cap - len(fidx), C), dtype=ml_dtypes.float8_e4m3fn)
    pad[:, 0] = -1.0
    xf_all = np.concatenate([xf_rows, pad], axis=0)
    xfs = np.ascontiguousarray(
        xf_all.reshape(NCORES, P, FN, C).transpose(0, 1, 3, 2)
    ).reshape(NCORES, P, C * FN)
    return xs, xfs, host_flag


def kernel(output=None, target=None, epoch=None):
    from concourse import bass_utils

    x = np.asarray(output)
    if x.dtype != np.float32:
        x = x.astype(np.float32)
    t32 = np.asarray(target).astype(np.int32)
    ep = int(np.asarray(epoch))
    assert x.shape == (B, C) and t32.shape == (B,)

    xs, xfs, host_flag = _prep_inputs(x, t32)
    in_maps = [{"x": xs[i], "xf": xfs[i]} for i in range(NCORES)]
    nc = _get_nc()
    res = bass_utils.run_bass_kernel_spmd(nc, in_maps, core_ids=list(range(NCORES)))

    lse_sum = 0.0
    g_sum = 0.0
    flg = 1.0 if host_flag else 0.0
    for rmap in res.results:
        o = rmap["out"].astype(np.float64)
        lse_sum += o[:, 0:NPAIR].sum()
        flg += o[:, 4].sum()
        g_sum += o[:, 5].sum()

    init_loss = (lse_sum - g_sum) / B
    corr = (float(ep) ** -0.65) / (4.0 ** -3) + 0.01
    loss = init_loss + (corr if flg > 0 else 0.0)
    return np.array(loss, dtype=np.float32)


# revision 7
# speedup vs baseline: 1.9937x; 1.0297x over previous
"""Trainium2 Bass kernel for nn_CustomLoss_47931835023913.

Computes: loss = mean_i( ln(sum_j exp(x_ij)) - x[i, target_i] )
                 + ((epoch**-0.65)*64 + 0.01) if any(target==2 & argmax==3)

v2 strategy (fp8 shipping, three-engine exp, DoubleRow row-sums):
  * Host-side LAYOUT prep only (rotation + dtype cast + compaction); all
    O(B) arithmetic runs on the NeuronCores.
      - rows rotated so column 0 holds x[i, target_i] (CE gather becomes
        a column sum); cast to fp8 e4m3 (CE mean over 4.2M rows gives a
        ~0.5 abs tolerance at rel 2e-2; e4m3 noise is zero-mean).
      - layout is tile-contiguous class-major [P, T, C, TN] so every
        engine reads/writes contiguous runs (the v1 kernel's strided
        matmul moving operands were the top bottleneck).
      - rows with target==2 are compacted into a separate small array
        (col 0 = x[i,3]) so the argmax flag only processes ~10% of rows
        and no mask ships.
  * Device per 512-row chunk:
      - exp into a shared e5m2 tile, split by contiguous class ranges
        across THREE engines: ScalarE native Exp, and DVE + GPSIMD via
        the Schraudolph bit trick (u8 = rint(x*4*log2e + 59.8) bitcast
        to e5m2 == 2^(x*log2e) with mean rel err +0.09%).
      - TensorE: 5 fp8 DoubleRow matmuls (stacked-identity weights) sum
        class pairs into one PSUM bank -> per-row sum(exp) in fp32.
      - TensorE: 1 plain fp8 matmul accumulates raw column 0 across all
        chunks into a persistent PSUM bank (the CE gather term).
      - ScalarE: Ln over a 2-chunk PSUM pair with accum_out.
  * Flag: DVE pairwise max tree over the compacted rows' cols 1..9,
    is_ge against col 0, accumulated count.
  * Host combines the 8 cores' [128, 6] accumulators in float64.
"""

import numpy as np

B = 4194304          # batch rows
C = 10               # classes
NCORES = 8
P = 128              # SBUF partitions
R = B // NCORES      # rows per core            = 524288
RP = R // P          # rows per partition       = 4096
TN = 512             # tile rows per partition
T = RP // TN         # chunks per core          = 8
NLN = 2              # chunks grouped per Ln (T // LNG)
LNG = T // NLN       # 4 chunks per Ln instruction

# exp class-range split points (elems within a 5120-elem chunk):
# ACT [0, SPLIT_A), DVE [SPLIT_A, SPLIT_D), GPSIMD [SPLIT_D, 5120).
# Balanced so each engine lands at ~17.5us including its side work
# (ACT: 2 Lns; DVE: flag tree + final accums; GPSIMD: exp only at
# 1.39ns/elem due to the 0.6 Q7 software efficiency).
SPLIT_A = 1984
SPLIT_D = 3520

FN = 416             # flag rows per partition; 8*128*416 = 425984 total
                     # capacity vs E[count]=419430, sd~614 (+10.7 sd)

SCH_A = float(4.0 * np.log2(np.e))
SCH_B = 59.8         # 60 (e5m2 bias*4) - 0.2 interp-bias centering

_CACHE = {}

_ACT_SET = "natural_log_exp_and_others"


def _pin_act_tables():
    import concourse.bacc as bacc_mod

    if getattr(bacc_mod.get_activation_tables, "_pinned", False):
        return
    orig = bacc_mod.get_activation_tables

    def pinned(module_arch):
        tables = orig(module_arch)
        return {
            name: (funcs if name == _ACT_SET else set())
            for name, funcs in tables.items()
        }

    pinned._pinned = True
    bacc_mod.get_activation_tables = pinned


def _build_nc():
    import ml_dtypes
    import concourse.mybir as mybir
    from concourse.bacc import Bacc
    from concourse.tile import TileContext

    _pin_act_tables()

    A = mybir.AluOpType
    F = mybir.ActivationFunctionType
    f32 = mybir.dt.float32
    e4 = mybir.dt.float8e4
    e5 = mybir.dt.float8e5
    u8 = mybir.dt.uint8
    CH = C * TN                       # elems per chunk per partition

    nc = Bacc("TRN2")
    x_d = nc.dram_tensor("x", [P, T * CH], e4, kind="ExternalInput")
    xf_d = nc.dram_tensor("xf", [P, C * FN], e4, kind="ExternalInput")
    out_d = nc.dram_tensor("out", [P, 6], f32, kind="ExternalOutput")

    # stacked identity for DoubleRow ([P, 2, P] as flat [P, 2P]) in e5m2,
    # plus a plain e4m3 identity for the raw-x gather matmul
    ident2_d = nc.inline_tensor(
        np.broadcast_to(
            np.eye(P, dtype=ml_dtypes.float8_e5m2)[:, None, :], (P, 2, P)
        ).reshape(P, 2 * P).copy(),
        name="ident2",
    )
    ident4_d = nc.inline_tensor(
        np.eye(P, dtype=ml_dtypes.float8_e4m3fn), name="ident4"
    )

    with TileContext(nc) as tc:
        with (
            tc.tile_pool(name="persist", bufs=1) as pp,
            tc.tile_pool(name="io", bufs=4) as iop,
            tc.tile_pool(name="work", bufs=4) as wp,
            tc.tile_pool(name="lnp", bufs=2) as lnp,
            tc.tile_pool(name="ps", bufs=1, space="PSUM") as psp,
            tc.tile_pool(name="psg", bufs=1, space="PSUM") as psgp,
        ):
            idt2 = pp.tile([P, 2 * P], e5)
            idt4 = pp.tile([P, P], e4)
            xf = pp.tile([P, C * FN], e4)
            acc = pp.tile([P, 6], f32)
            nc.sync.dma_start(idt2[:], ident2_d[:])
            nc.sync.dma_start(idt4[:], ident4_d[:])
            nc.sync.dma_start(xf[:], xf_d[:])
            idt2v = idt2.rearrange("p (a b) -> p a b", a=2)

            psg = psgp.tile([P, TN], f32, tag="g", name="psg")
            xfv = xf.rearrange("p (c n) -> p c n", c=C)

            # flag-tree ops, issued one-per-chunk into DVE's dependency
            # gaps instead of as a serial tail
            m1 = wp.tile([P, 4 * FN], e4, tag="m1", name="m1", bufs=1)
            m1v = m1.rearrange("p (c n) -> p c n", c=4)
            m2 = wp.tile([P, 2 * FN], e4, tag="m2", name="m2", bufs=1)
            m2v = m2.rearrange("p (c n) -> p c n", c=2)
            m3 = wp.tile([P, FN], e4, tag="m3", name="m3", bufs=1)
            m4 = wp.tile([P, FN], e4, tag="m4", name="m4", bufs=1)
            ge = wp.tile([P, FN], f32, tag="ge", name="ge", bufs=1)

            def flag_step(k):
                if k == 0:
                    nc.vector.tensor_tensor(
                        m1v, xfv[:, 1:5, :], xfv[:, 5:9, :], A.max
                    )
                elif k == 1:
                    nc.vector.tensor_tensor(
                        m2v, m1v[:, 0:2, :], m1v[:, 2:4, :], A.max
                    )
                elif k == 2:
                    nc.vector.tensor_tensor(
                        m3[:], m2v[:, 0, :], m2v[:, 1, :], A.max
                    )
                elif k == 3:
                    nc.vector.tensor_tensor(m4[:], m3[:], xfv[:, 9, :], A.max)
                elif k == 4:
                    nc.vector.scalar_tensor_tensor(
                        ge[:], xfv[:, 0, :], 1.0, m4[:], A.mult, A.is_ge,
                        accum_out=acc[:, 4:5],
                    )

            s_grp = None
            for t in range(T):
                x_t = iop.tile([P, CH], e4, tag="x", name="x_t")
                nc.sync.dma_start(x_t[:], x_d[:, t * CH : (t + 1) * CH])

                e_t = wp.tile([P, CH], u8, tag="e", name="e_t")
                e5v = e_t.bitcast(e5)
                # three-engine exp, contiguous splits
                nc.scalar.activation(
                    e5v[:, 0:SPLIT_A], x_t[:, 0:SPLIT_A], F.Exp
                )
                nc.vector.tensor_scalar(
                    e_t[:, SPLIT_A:SPLIT_D], x_t[:, SPLIT_A:SPLIT_D],
                    SCH_A, SCH_B, A.mult, A.add,
                )
                flag_step(t - 1)
                nc.gpsimd.tensor_scalar(
                    e_t[:, SPLIT_D:CH], x_t[:, SPLIT_D:CH],
                    SCH_A, SCH_B, A.mult, A.add,
                )

                # row sums: 5 DoubleRow matmuls accumulate class pairs
                grp, sub = divmod(t, LNG)
                if sub == 0:
                    s_grp = psp.tile([P, LNG * TN], f32, tag="s", name="s_grp")
                s_ps = s_grp[:, sub * TN : (sub + 1) * TN]
                ev = e5v.rearrange("p (c n) -> p c n", c=C)
                for cc in range(C // 2):
                    nc.tensor.matmul(
                        s_ps, idt2v, ev[:, 2 * cc : 2 * cc + 2, :],
                        start=(cc == 0), stop=(cc == C // 2 - 1),
                        perf_mode=mybir.MatmulPerfMode.DoubleRow,
                        skip_group_check=True,
                    )

                # gather: accumulate raw column 0 across chunks (plain fp8)
                nc.tensor.matmul(
                    psg[:], idt4[:], x_t[:, 0:TN],
                    start=(t == 0), stop=(t == T - 1),
                    skip_group_check=True,
                )

                if sub == LNG - 1:
                    lnscr = lnp.tile([P, LNG * TN], f32, tag="ln", name="lnscr")
                    nc.scalar.activation(
                        lnscr[:], s_grp[:], F.Ln,
                        accum_out=acc[:, grp : grp + 1],
                    )

            # gather total
            gscr = wp.tile([P, TN], f32, tag="gs", name="gscr", bufs=1)
            nc.vector.tensor_scalar(
                gscr[:], psg[:], 1.0, 0.0, A.mult, A.add,
                accum_out=acc[:, 5:6],
            )

            nc.sync.dma_start(out_d[:], acc[:])
    nc.finalize()
    return nc


def _get_nc():
    if "nc" not in _CACHE:
        _CACHE["nc"] = _build_nc()
    return _CACHE["nc"]


def _prep_inputs(x, t32):
    """Rotate rows by target, cast fp8, tile-contiguous class-major layout;
    compact target==2 rows (col 0 = x[:,3]) for the flag path."""
    import ml_dtypes

    idx = (t32[:, None] + np.arange(C, dtype=np.int32)[None, :]) % C
    xr = np.take_along_axis(x, idx, axis=1).astype(ml_dtypes.float8_e4m3fn)
    # [B, C] -> [cores, P, T, TN, C] -> [cores, P, T, C, TN]
    xs = np.ascontiguousarray(
        xr.reshape(NCORES, P, T, TN, C).transpose(0, 1, 2, 4, 3)
    ).reshape(NCORES, P, T * C * TN)

    fidx = np.flatnonzero(t32 == 2)
    nf_cap = NCORES * P * FN
    host_flag = False
    if len(fidx) > nf_cap:
        # overflow beyond device capacity: fold the excess on host
        # (never triggers for randn inputs; correctness backstop)
        extra = fidx[nf_cap:]
        host_flag = bool(
            np.any(np.argmax(x[extra], axis=1) == 3)
        )
        fidx = fidx[:nf_cap]
    xf_rows = x[fidx][:, [3, 4, 5, 6, 7, 8, 9, 0, 1, 2]].astype(
        ml_dtypes.float8_e4m3fn
    )
    pad = np.zeros((nf_cap - len(fidx), C), dtype=ml_dtypes.float8_e4m3fn)
    pad[:, 0] = -1.0
    xf_all = np.concatenate([xf_rows, pad], axis=0)
    xfs = np.ascontiguousarray(
        xf_all.reshape(NCORES, P, FN, C).transpose(0, 1, 3, 2)
    ).reshape(NCORES, P, C * FN)
    return xs, xfs, host_flag


def kernel(output=None, target=None, epoch=None):
    from concourse import bass_utils

    x = np.asarray(output)
    if x.dtype != np.float32:
        x = x.astype(np.float32)
    t32 = np.asarray(target).astype(np.int32)
    ep = int(np.asarray(epoch))
    assert x.shape == (B, C) and t32.shape == (B,)

    xs, xfs, host_flag = _prep_inputs(x, t32)
    in_maps = [{"x": xs[i], "xf": xfs[i]} for i in range(NCORES)]
    nc = _get_nc()
    res = bass_utils.run_bass_kernel_spmd(nc, in_maps, core_ids=list(range(NCORES)))

    lse_sum = 0.0
    g_sum = 0.0
    flg = 1.0 if host_flag else 0.0
    for rmap in res.results:
        o = rmap["out"].astype(np.float64)
        lse_sum += o[:, 0:NLN].sum()
        flg += o[:, 4].sum()
        g_sum += o[:, 5].sum()

    init_loss = (lse_sum - g_sum) / B
    corr = (float(ep) ** -0.65) / (4.0 ** -3) + 0.01
    loss = init_loss + (corr if flg > 0 else 0.0)
    return np.array(loss, dtype=np.float32)


# revision 13
# speedup vs baseline: 2.0862x; 1.0464x over previous
"""Trainium2 Bass kernel for nn_CustomLoss_47931835023913.

Computes: loss = mean_i( ln(sum_j exp(x_ij)) - x[i, target_i] )
                 + ((epoch**-0.65)*64 + 0.01) if any(target==2 & argmax==3)

v2 strategy (fp8 shipping, three-engine exp, DoubleRow row-sums):
  * Host-side LAYOUT prep only (rotation + dtype cast + compaction); all
    O(B) arithmetic runs on the NeuronCores.
      - rows rotated so column 0 holds x[i, target_i] (CE gather becomes
        a column sum); cast to fp8 e4m3 (CE mean over 4.2M rows gives a
        ~0.5 abs tolerance at rel 2e-2; e4m3 noise is zero-mean).
      - layout is tile-contiguous class-major [P, T, C, TN] so every
        engine reads/writes contiguous runs (the v1 kernel's strided
        matmul moving operands were the top bottleneck).
      - rows with target==2 are compacted into a separate small array
        (col 0 = x[i,3]) so the argmax flag only processes ~10% of rows
        and no mask ships.
  * Device per 512-row chunk:
      - exp into a shared e5m2 tile, split by contiguous class ranges
        across THREE engines: ScalarE native Exp, and DVE + GPSIMD via
        the Schraudolph bit trick (u8 = rint(x*4*log2e + 59.8) bitcast
        to e5m2 == 2^(x*log2e) with mean rel err +0.09%).
      - TensorE: 5 fp8 DoubleRow matmuls (stacked-identity weights) sum
        class pairs into one PSUM bank -> per-row sum(exp) in fp32.
      - TensorE: 1 plain fp8 matmul accumulates raw column 0 across all
        chunks into a persistent PSUM bank (the CE gather term).
      - ScalarE: Ln over a 2-chunk PSUM pair with accum_out.
  * Flag: DVE pairwise max tree over the compacted rows' cols 1..9,
    is_ge against col 0, accumulated count.
  * Host combines the 8 cores' [128, 6] accumulators in float64.
"""

import numpy as np

B = 4194304          # batch rows
C = 10               # classes
NCORES = 8
P = 128              # SBUF partitions
R = B // NCORES      # rows per core            = 524288
RP = R // P          # rows per partition       = 4096
TN = 512             # tile rows per partition
T = RP // TN         # chunks per core          = 8
NLN = 3              # Ln accumulator columns (chunk groups 0-3, 4-6, 7)
LNG = 4              # chunks per PSUM tile generation

# exp class-range split points (elems within a 5120-elem chunk):
# ACT [0, SPLIT_A), DVE [SPLIT_A, SPLIT_D), GPSIMD [SPLIT_D, 5120).
# Balanced so each engine lands at ~17.5us including its side work
# (ACT: 2 Lns; DVE: flag tree + final accums; GPSIMD: exp only at
# 1.39ns/elem due to the 0.6 Q7 software efficiency).
SPLIT_A = 1984
SPLIT_D = 3520

FN = 416             # flag rows per partition; 8*128*416 = 425984 total
                     # capacity vs E[count]=419430, sd~614 (+10.7 sd)

SCH_A = float(4.0 * np.log2(np.e))
SCH_B = 59.8         # 60 (e5m2 bias*4) - 0.2 interp-bias centering

_CACHE = {}

_ACT_SET = "natural_log_exp_and_others"


def _pin_act_tables():
    import concourse.bacc as bacc_mod

    if getattr(bacc_mod.get_activation_tables, "_pinned", False):
        return
    orig = bacc_mod.get_activation_tables

    def pinned(module_arch):
        tables = orig(module_arch)
        return {
            name: (funcs if name == _ACT_SET else set())
            for name, funcs in tables.items()
        }

    pinned._pinned = True
    bacc_mod.get_activation_tables = pinned


def _build_nc():
    import ml_dtypes
    import concourse.mybir as mybir
    from concourse.bacc import Bacc
    from concourse.tile import TileContext

    _pin_act_tables()

    A = mybir.AluOpType
    F = mybir.ActivationFunctionType
    f32 = mybir.dt.float32
    e4 = mybir.dt.float8e4
    e5 = mybir.dt.float8e5
    u8 = mybir.dt.uint8
    CH = C * TN                       # elems per chunk per partition

    nc = Bacc("TRN2")
    x_d = nc.dram_tensor("x", [P, T * CH], e4, kind="ExternalInput")
    xf_d = nc.dram_tensor("xf", [P, C * FN], e4, kind="ExternalInput")
    out_d = nc.dram_tensor("out", [P, 6], f32, kind="ExternalOutput")

    # stacked identity for DoubleRow ([P, 2, P] as flat [P, 2P]) in e5m2,
    # plus a plain e4m3 identity for the raw-x gather matmul
    ident2_d = nc.inline_tensor(
        np.broadcast_to(
            np.eye(P, dtype=ml_dtypes.float8_e5m2)[:, None, :], (P, 2, P)
        ).reshape(P, 2 * P).copy(),
        name="ident2",
    )
    ident4_d = nc.inline_tensor(
        np.eye(P, dtype=ml_dtypes.float8_e4m3fn), name="ident4"
    )

    with TileContext(nc) as tc:
        with (
            tc.tile_pool(name="persist", bufs=1) as pp,
            tc.tile_pool(name="io", bufs=T) as iop,
            tc.tile_pool(name="work", bufs=4) as wp,
            tc.tile_pool(name="lnp", bufs=2) as lnp,
            tc.tile_pool(name="ps", bufs=1, space="PSUM") as psp,
            tc.tile_pool(name="psg", bufs=1, space="PSUM") as psgp,
        ):
            # x-chunk DMAs go first on the SP queue so HBM streaming starts
            # as early as possible; idents/flag rows ride behind them (their
            # first consumers run microseconds later)
            idt2 = pp.tile([P, 2 * P], e5)
            idt4 = pp.tile([P, P], e4)
            xf = pp.tile([P, C * FN], e4)
            acc = pp.tile([P, 6], f32)
            x_ts = []
            for t in range(T):
                x_t = iop.tile([P, CH], e4, tag="x", name="x_t")
                nc.sync.dma_start(x_t[:], x_d[:, t * CH : (t + 1) * CH])
                x_ts.append(x_t)
                if t == 2:
                    # idents + flag rows ride behind the first three x
                    # chunks; their consumers run much later
                    nc.sync.dma_start(idt2[:], ident2_d[:])
                    nc.sync.dma_start(idt4[:], ident4_d[:])
                    nc.sync.dma_start(xf[:], xf_d[:])
            idt2v = idt2.rearrange("p (a b) -> p a b", a=2)

            psg = psgp.tile([P, TN], f32, tag="g", name="psg")
            xfv = xf.rearrange("p (c n) -> p c n", c=C)

            # flag-tree ops, issued one-per-chunk into DVE's dependency
            # gaps instead of as a serial tail
            m1 = wp.tile([P, 4 * FN], e4, tag="m1", name="m1", bufs=1)
            m1v = m1.rearrange("p (c n) -> p c n", c=4)
            m2 = wp.tile([P, 2 * FN], e4, tag="m2", name="m2", bufs=1)
            m2v = m2.rearrange("p (c n) -> p c n", c=2)
            m3 = wp.tile([P, FN], e4, tag="m3", name="m3", bufs=1)
            m4 = wp.tile([P, FN], e4, tag="m4", name="m4", bufs=1)
            ge = wp.tile([P, FN], f32, tag="ge", name="ge", bufs=1)

            def flag_step(k):
                if k == 0:
                    nc.vector.tensor_tensor(
                        m1v, xfv[:, 1:5, :], xfv[:, 5:9, :], A.max
                    )
                elif k == 1:
                    nc.vector.tensor_tensor(
                        m2v, m1v[:, 0:2, :], m1v[:, 2:4, :], A.max
                    )
                elif k == 2:
                    nc.vector.tensor_tensor(
                        m3[:], m2v[:, 0, :], m2v[:, 1, :], A.max
                    )
                elif k == 3:
                    nc.vector.tensor_tensor(m4[:], m3[:], xfv[:, 9, :], A.max)
                elif k == 4:
                    nc.vector.scalar_tensor_tensor(
                        ge[:], xfv[:, 0, :], 1.0, m4[:], A.mult, A.is_ge,
                        accum_out=acc[:, 4:5],
                    )

            s_grp = None
            for t in range(T):
                x_t = x_ts[t]
                e_t = wp.tile([P, CH], u8, tag="e", name="e_t")
                e5v = e_t.bitcast(e5)
                # three-engine exp, contiguous splits
                nc.scalar.activation(
                    e5v[:, 0:SPLIT_A], x_t[:, 0:SPLIT_A], F.Exp
                )
                nc.vector.tensor_scalar(
                    e_t[:, SPLIT_A:SPLIT_D], x_t[:, SPLIT_A:SPLIT_D],
                    SCH_A, SCH_B, A.mult, A.add,
                )
                flag_step(t - 1)
                nc.gpsimd.tensor_scalar(
                    e_t[:, SPLIT_D:CH], x_t[:, SPLIT_D:CH],
                    SCH_A, SCH_B, A.mult, A.add,
                )

                # row sums: 5 DoubleRow matmuls accumulate class pairs
                grp, sub = divmod(t, LNG)
                if sub == 0:
                    s_grp = psp.tile([P, LNG * TN], f32, tag="s", name="s_grp")
                s_ps = s_grp[:, sub * TN : (sub + 1) * TN]
                ev = e5v.rearrange("p (c n) -> p c n", c=C)
                for cc in range(C // 2):
                    nc.tensor.matmul(
                        s_ps, idt2v, ev[:, 2 * cc : 2 * cc + 2, :],
                        start=(cc == 0), stop=(cc == C // 2 - 1),
                        perf_mode=mybir.MatmulPerfMode.DoubleRow,
                        skip_group_check=True,
                    )

                # gather: accumulate raw column 0 across chunks (plain fp8)
                nc.tensor.matmul(
                    psg[:], idt4[:], x_t[:, 0:TN],
                    start=(t == 0), stop=(t == T - 1),
                    skip_group_check=True,
                )

                # Lns: (chunks 0-3) -> col 0, (4-6) -> col 1, (7) -> col 2.
                # Splitting the last group keeps the tail Ln small.
                if t == 3:
                    lnscr = lnp.tile([P, LNG * TN], f32, tag="ln", name="lnscr")
                    nc.scalar.activation(
                        lnscr[:], s_grp[:], F.Ln, accum_out=acc[:, 0:1]
                    )
                elif t == 6:
                    lnscr = lnp.tile([P, LNG * TN], f32, tag="ln", name="lnscr")
                    nc.scalar.activation(
                        lnscr[:, 0 : 3 * TN], s_grp[:, 0 : 3 * TN], F.Ln,
                        accum_out=acc[:, 1:2],
                    )
                elif t == 7:
                    lnscr = lnp.tile([P, LNG * TN], f32, tag="ln", name="lnscr")
                    nc.scalar.activation(
                        lnscr[:, 0:TN], s_grp[:, 3 * TN : 4 * TN], F.Ln,
                        accum_out=acc[:, 2:3],
                    )

            # gather total
            gscr = wp.tile([P, TN], f32, tag="gs", name="gscr", bufs=1)
            nc.vector.tensor_scalar(
                gscr[:], psg[:], 1.0, 0.0, A.mult, A.add,
                accum_out=acc[:, 5:6],
            )

            nc.sync.dma_start(out_d[:], acc[:])
    nc.finalize()
    return nc


def _get_nc():
    if "nc" not in _CACHE:
        _CACHE["nc"] = _build_nc()
    return _CACHE["nc"]


def _prep_inputs(x, t32):
    """Rotate rows by target, cast fp8, tile-contiguous class-major layout;
    compact target==2 rows (col 0 = x[:,3]) for the flag path."""
    import ml_dtypes

    idx = (t32[:, None] + np.arange(C, dtype=np.int32)[None, :]) % C
    xr = np.take_along_axis(x, idx, axis=1).astype(ml_dtypes.float8_e4m3fn)
    # [B, C] -> [cores, P, T, TN, C] -> [cores, P, T, C, TN]
    xs = np.ascontiguousarray(
        xr.reshape(NCORES, P, T, TN, C).transpose(0, 1, 2, 4, 3)
    ).reshape(NCORES, P, T * C * TN)

    fidx = np.flatnonzero(t32 == 2)
    nf_cap = NCORES * P * FN
    host_flag = False
    if len(fidx) > nf_cap:
        # overflow beyond device capacity: fold the excess on host
        # (never triggers for randn inputs; correctness backstop)
        extra = fidx[nf_cap:]
        host_flag = bool(
            np.any(np.argmax(x[extra], axis=1) == 3)
        )
        fidx = fidx[:nf_cap]
    xf_rows = x[fidx][:, [3, 4, 5, 6, 7, 8, 9, 0, 1, 2]].astype(
        ml_dtypes.float8_e4m3fn
    )
    pad = np.zeros((nf_cap - len(fidx), C), dtype=ml_dtypes.float8_e4m3fn)
    pad[:, 0] = -1.0
    xf_all = np.concatenate([xf_rows, pad], axis=0)
    xfs = np.ascontiguousarray(
        xf_all.reshape(NCORES, P, FN, C).transpose(0, 1, 3, 2)
    ).reshape(NCORES, P, C * FN)
    return xs, xfs, host_flag


def kernel(output=None, target=None, epoch=None):
    from concourse import bass_utils

    x = np.asarray(output)
    if x.dtype != np.float32:
        x = x.astype(np.float32)
    t32 = np.asarray(target).astype(np.int32)
    ep = int(np.asarray(epoch))
    assert x.shape == (B, C) and t32.shape == (B,)

    xs, xfs, host_flag = _prep_inputs(x, t32)
    in_maps = [{"x": xs[i], "xf": xfs[i]} for i in range(NCORES)]
    nc = _get_nc()
    res = bass_utils.run_bass_kernel_spmd(nc, in_maps, core_ids=list(range(NCORES)))

    lse_sum = 0.0
    g_sum = 0.0
    flg = 1.0 if host_flag else 0.0
    for rmap in res.results:
        o = rmap["out"].astype(np.float64)
        lse_sum += o[:, 0:NLN].sum()
        flg += o[:, 4].sum()
        g_sum += o[:, 5].sum()

    init_loss = (lse_sum - g_sum) / B
    corr = (float(ep) ** -0.65) / (4.0 ** -3) + 0.01
    loss = init_loss + (corr if flg > 0 else 0.0)
    return np.array(loss, dtype=np.float32)
